# revision 1
# baseline (speedup 1.0000x reference)
"""DSP feature extractor on 8 trn2 NeuronCores.

Self-contained: kernel(x) -> [131072, 18] f32.

Design (per core, 16384 rows x 1000 f32):
- rfft via 2-stage Cooley-Tukey (1000 = 4*250) as PE matmuls in float32r:
  n = 4*n1 + n2. Stage 1: DFT-250 per n2-subsequence (PE-transposed input,
  stationary DFT matrices). Stage 2: twiddle+DFT-4 recombination via
  per-chunk stationaries with conjugate-symmetry folded in.
- power spectrum, weighted bin sums (total/hr/rr/centroid/bandwidth/lag-cos),
  boundary taps (x0,x1,x998,x999,S1) via further PE matmuls.
- time-domain row scans on ACT/DVE/GPSIMD: x^2 (gpsimd), Sign (ACT),
  S4=sum(x^2)^2 (ACT square+accum), S3 (DVE affine_mul_reduce),
  zero-cross pair sum (DVE amr on signs), min/max (DVE reduce).
- per-bin ps shipped to host; host does argmax (+np.fft refinement of rows
  whose top-2 gap < 1%), assembles the 18 features from the primitives.
"""
import functools
import numpy as np

L = 1000
N1, N2 = 250, 4
NB = 126          # stored bins per subsequence (DFT-250 of real input)
RPC = 16384       # rows per core
NCORE = 8
NBLK = 32         # blocks of 512 rows per core
EPS = 1e-8

# ---------------------------------------------------------------- constants

def _chunk_bins(c):
    return [k for k in range(501)
            if 16 * c <= min(k % N1, (N1 - k % N1) % N1) < 16 * c + 16]


def _build_f1():
    n1 = np.arange(N1)
    out = {}
    for n2 in range(N2):
        full = np.zeros((N1, 256), np.float64)
        for j in range(NB):
            full[:, 2 * j] = np.cos(2 * np.pi * j * n1 / N1)
            full[:, 2 * j + 1] = -np.sin(2 * np.pi * j * n1 / N1)
        full[0 if n2 < 2 else 249, 252] = 1.0  # boundary tap
        for kc in range(2):
            for oc in range(2):
                M = np.zeros((128, 128))
                blk = full[kc * 125:(kc + 1) * 125, oc * 126:oc * 126 + 128]
                M[:125, :blk.shape[1]] = blk
                out[(n2, kc, oc)] = M
    return out


def _build_t2():
    blocks = {}
    for c in range(8):
        ks = _chunk_bins(c)
        for n2 in range(N2):
            pieces = {0: np.zeros((128, 128)), 1: np.zeros((128, 128))}
            for i, k in enumerate(ks):
                k1 = k % N1
                k1s = min(k1, (N1 - k1) % N1)
                conj = k1 >= NB
                co = np.cos(2 * np.pi * n2 * k / L)
                si = -np.sin(2 * np.pi * n2 * k / L)
                ims = -1.0 if conj else 1.0
                for (ch, wre, wim) in ((2 * k1s, co, si),
                                       (2 * k1s + 1, -si * ims, co * ims)):
                    oc = 0 if ch < 126 else 1
                    lc = ch - oc * 126
                    pieces[oc][lc, 2 * i] += wre
                    pieces[oc][lc, 2 * i + 1] += wim
            plist = []
            for oc in range(2):
                nz = np.nonzero(pieces[oc].any(axis=1))[0]
                if len(nz):
                    lo, hi = int(nz.min()), int(nz.max() + 1)
                    plist.append((oc, lo, hi, pieces[oc][lo:hi]))
            blocks[(c, n2)] = plist
    return blocks


def _build_weights():
    k = np.arange(501)
    f = k * 0.1
    coef = np.where((k == 0) | (k == 500), 1.0, 2.0)
    ws = [np.ones(501),
          ((k >= 8) & (k <= 30)).astype(float),
          ((k >= 1) & (k <= 5)).astype(float),
          f, f * f,
          np.cos(2 * np.pi * k / L) * coef / L,
          np.cos(4 * np.pi * k / L) * coef / L]
    mats = []
    for c in range(8):
        ks = _chunk_bins(c)
        M = np.zeros((128, 8))
        for i, k_ in enumerate(ks):
            for col, w in enumerate(ws):
                M[2 * i, col] = w[k_]
                M[2 * i + 1, col] = w[k_]
        mats.append(M)
    return mats


def _build_pairs():
    mats = []
    for c in range(8):
        nb = len(_chunk_bins(c))
        M = np.zeros((128, 64))
        for i in range(nb):
            M[2 * i, i] = 1.0
            M[2 * i + 1, i] = 1.0
        mats.append(M)
    return mats


def _pack_consts():
    """Pack all stationaries column-wise into one [128, NC] f32 array.

    Returns (array, meta) where meta maps names -> (col, width) and T2 meta
    is a per-chunk list of (n2, oc, lo, hi, col, width).
    """
    cols = []
    meta = {}
    off = 0

    def push(name, M, width=None, row0=0):
        nonlocal off
        w = M.shape[1] if width is None else width
        A = np.zeros((128, w), np.float32)
        A[row0:row0 + M.shape[0], :M.shape[1]] = M
        cols.append(A)
        meta[name] = (off, w)
        off += w

    push("ident", np.eye(128))
    F1 = _build_f1()
    for key, M in F1.items():
        push(("f1",) + key, M)
    T2 = _build_t2()
    t2meta = {}
    for c in range(8):
        w = 2 * len(_chunk_bins(c))
        lst = []
        for n2 in range(N2):
            for (oc, lo, hi, M) in T2[(c, n2)]:
                name = ("t2", c, n2, oc, lo)
                push(name, M[:, :w], width=w, row0=lo)
                lst.append((n2, oc, lo, hi, meta[name][0], w))
        t2meta[c] = lst
    for c, M in enumerate(_build_pairs()):
        push(("pw", c), M)
    for c, M in enumerate(_build_weights()):
        push(("ws", c), M)
    # extras: for each n2: oc0 tap (ch0 -> col4 S1), oc1 tap (lc126 -> col n2)
    for n2 in range(N2):
        E = np.zeros((1, 8))
        E[0, 4] = 1.0
        push(("exA", n2), E)
        E = np.zeros((1, 8))
        E[0, n2] = 1.0
        push(("exB", n2), E, row0=126)
    arr = np.concatenate(cols, axis=1).astype(np.float32)
    return arr, meta, t2meta


# ---------------------------------------------------------------- device

def _devk(consts_meta, t2meta, nc, x, consts):
    import concourse.tile as tile
    from concourse import mybir
    F32 = mybir.dt.float32
    F32R = mybir.dt.float32r
    ALU = mybir.AluOpType
    AF = mybir.ActivationFunctionType
    AX = mybir.AxisListType
    NC = consts.shape[1]

    ps_out = nc.dram_tensor("ps_out", [512, RPC], F32, kind="ExternalOutput")
    st_out = nc.dram_tensor("st_out", [80, RPC], F32, kind="ExternalOutput")
    acc_out = nc.dram_tensor("acc_out", [128, 1024], F32, kind="ExternalOutput")

    def cslice(t, name):
        o, w = consts_meta[name]
        return t[:, o:o + w]

    with tile.TileContext(nc) as tc:
        with tc.tile_pool(name="P", bufs=1) as P, \
             tc.tile_pool(name="PS", bufs=1, space="PSUM") as PS:
            ident = P.tile([128, 128], F32)
            CR = P.tile([128, NC], F32R)
            stage = P.tile([128, 512], F32)
            # stage consts -> f32r
            for o in range(0, NC, 512):
                w = min(512, NC - o)
                nc.sync.dma_start(stage[:, :w], consts[:, o:o + w])
                nc.vector.tensor_copy(CR[:, o:o + w], stage[:, :w])
            nc.sync.dma_start(ident[:], consts[:, 0:128])

            xsb = [P.tile([128, 4096], F32, name=f"xsb{i}") for i in range(2)]
            x2sb = [P.tile([128, 4096], F32, name=f"x2sb{i}") for i in range(2)]
            ssb = [P.tile([128, 4096], F32, name=f"ssb{i}") for i in range(2)]
            xT = P.tile([128, 4096], F32R)       # 8 x (n2,kc) [128,512]
            Ysb = P.tile([128, 4096], F32R)      # 8 x (n2,oc) [128,512]
            sqsb = P.tile([128, 1024], F32R)     # 2 chunks
            pssb = [P.tile([128, 512], F32, name=f"pssb{i}") for i in range(2)]
            stsb = [P.tile([80, 512], F32, name=f"stsb{i}") for i in range(2)]
            acc = P.tile([128, 1024], F32)
            scr = [P.tile([128, 1024], F32, name=f"scr{i}") for i in range(2)]
            dummy = P.tile([128, 1], F32)
            for t in xsb:
                nc.vector.memset(t[:], 0.0)
            nc.vector.memset(xT[:].bitcast(mybir.dt.uint32), 0)
            nc.vector.memset(sqsb[:].bitcast(mybir.dt.uint32), 0)

            for b in range(NBLK):
                p = b % 2
                xb, x2b, sb = xsb[p], x2sb[p], ssb[p]
                for q in range(4):
                    nc.sync.dma_start(
                        xb[:, q * 1024:q * 1024 + 1000],
                        x[b * 512 + q * 128:b * 512 + (q + 1) * 128, :])
                nc.gpsimd.tensor_tensor(x2b[:], xb[:], xb[:], ALU.mult)
                nc.scalar.activation(sb[:], xb[:], AF.Sign)
                for q in range(4):
                    t = b * 4 + q
                    sl = slice(q * 1024, q * 1024 + 1000)
                    nc.vector.affine_mul_reduce(
                        out=dummy.broadcast_to([128, 1000]),
                        accum_out=acc[:, t * 8:t * 8 + 1],
                        in0=x2b[:, sl], in1=xb[:, sl], scale=1.0, bias=0.0)
                    nc.vector.affine_mul_reduce(
                        out=dummy.broadcast_to([128, 999]),
                        accum_out=acc[:, t * 8 + 1:t * 8 + 2],
                        in0=sb[:, q * 1024 + 1:q * 1024 + 1000],
                        in1=sb[:, q * 1024:q * 1024 + 999], scale=1.0, bias=0.0)
                    nc.vector.tensor_reduce(acc[:, t * 8 + 2:t * 8 + 3],
                                            xb[:, sl], axis=AX.X, op=ALU.min)
                    nc.vector.tensor_reduce(acc[:, t * 8 + 3:t * 8 + 4],
                                            xb[:, sl], axis=AX.X, op=ALU.max)
                    nc.scalar.activation(scr[q % 2][:, :1000], x2b[:, sl],
                                         AF.Square,
                                         accum_out=acc[:, t * 8 + 4:t * 8 + 5])
                # transposes -> xT (n2,kc), f32r
                for n2 in range(N2):
                    for kc in range(2):
                        trp = PS.tile([128, 512], F32, tag="trT", name="trp")
                        for q in range(4):
                            src = xb[:, q * 1024:(q + 1) * 1024].rearrange(
                                "p (a c) -> p c a", c=4)[:, n2,
                                kc * 125:kc * 125 + 125]
                            nc.tensor.transpose(
                                trp[0:125, q * 128:(q + 1) * 128], src, ident[:])
                        nc.scalar.copy(
                            xT[0:125, (n2 * 2 + kc) * 512:(n2 * 2 + kc + 1) * 512],
                            trp[0:125, :])
                # stage 1
                for n2 in range(N2):
                    for oc in range(2):
                        Yp = PS.tile([128, 512], F32, tag="Y", name="Yp")
                        for kc in range(2):
                            nc.tensor.matmul(
                                Yp[:], cslice(CR, ("f1", n2, kc, oc)),
                                xT[:, (n2 * 2 + kc) * 512:(n2 * 2 + kc + 1) * 512],
                                start=(kc == 0), stop=(kc == 1))
                        nc.scalar.copy(
                            Ysb[:, (n2 * 2 + oc) * 512:(n2 * 2 + oc + 1) * 512],
                            Yp[:])
                # stage 2 + sq + pairs + weights
                STp = PS.tile([128, 512], F32, tag="ST", name="STp")
                for c in range(8):
                    w = 2 * len(_chunk_bins(c))
                    Xp = PS.tile([128, 512], F32, tag="X", name="Xp")
                    pieces = t2meta[c]
                    for i, (n2, oc, lo, hi, col, ww) in enumerate(pieces):
                        nc.tensor.matmul(
                            Xp[0:w, :], CR[0:128, col:col + w],
                            Ysb[0:128, (n2 * 2 + oc) * 512:(n2 * 2 + oc + 1) * 512],
                            start=(i == 0), stop=(i == len(pieces) - 1))
                    sqt = sqsb[:, (c % 2) * 512:(c % 2 + 1) * 512]
                    nc.scalar.activation(sqt[0:w, :], Xp[0:w, :], AF.Square)
                    nb = w // 2
                    nc.tensor.matmul(
                        STp[0:8, :], CR[0:w, slice(*_ws_col(consts_meta, c))],
                        sqt[0:w, :], start=(c == 0), stop=(c == 7))
                    pp = PS.tile([64, 512], F32, tag="PSm", name="pp")
                    o_pw, _ = consts_meta[("pw", c)]
                    nc.tensor.matmul(
                        pp[0:nb, :], CR[0:w, o_pw:o_pw + nb],
                        sqt[0:w, :], start=True, stop=True)
                    nc.scalar.copy(
                        pssb[(c // 2) % 2][64 * (c % 2):64 * (c % 2) + nb, :],
                        pp[0:nb, :])
                    if c % 2 == 1:
                        nc.sync.dma_start(
                            ps_out[(c // 2) * 128:(c // 2 + 1) * 128,
                                   b * 512:(b + 1) * 512],
                            pssb[(c // 2) % 2][:])
                # extras
                EXp = PS.tile([32, 512], F32, tag="EX", name="EXp")
                exi = 0
                for n2 in range(N2):
                    for (nm, lo) in ((("exA", n2), 0), (("exB", n2), 126)):
                        o_e, _ = consts_meta[nm]
                        oc = 0 if nm[0] == "exA" else 1
                        nc.tensor.matmul(
                            EXp[0:8, :], CR[0:128, o_e:o_e + 8],
                            Ysb[0:128,
                                (n2 * 2 + oc) * 512:(n2 * 2 + oc + 1) * 512],
                            start=(exi == 0), stop=(exi == 7))
                        exi += 1
                nc.scalar.copy(stsb[p][0:8, :], STp[0:8, :])
                nc.scalar.copy(stsb[p][32:40, :], EXp[0:8, :])
                nc.sync.dma_start(st_out[0:40, b * 512:(b + 1) * 512],
                                  stsb[p][0:40, :])
            nc.sync.dma_start(acc_out[:], acc[:])
    return ps_out, st_out, acc_out


def _ws_col(meta, c):
    o, w = meta[("ws", c)]
    return o, o + 8


# ---------------------------------------------------------------- host

_CACHE = {}


def _get_compiled():
    if "fn" in _CACHE:
        return _CACHE["fn"]
    import jax
    from jax.sharding import Mesh, PartitionSpec as P
    from concourse.bass2jax import bass_jit, bass_shard_map

    consts, meta, t2meta = _pack_consts()
    devs = jax.devices()[:NCORE]
    mesh = Mesh(np.array(devs), ("d",))
    fn = bass_shard_map(
        bass_jit(functools.partial(_devk, meta, t2meta)),
        mesh=mesh,
        in_specs=(P("d"), P()),
        out_specs=(P("d"), P("d"), P("d")),
    )
    _CACHE["fn"] = (fn, consts)
    return _CACHE["fn"]


def _features_host(prim):
    S1, S2, S3, S4 = prim["S1"], prim["S2"], prim["S3"], prim["S4"]
    mn, mx, Sss = prim["min"], prim["max"], prim["Sss"]
    tp, hr, rr, Sf, Sf2 = prim["tp"], prim["hr"], prim["rr"], prim["Sf"], prim["Sf2"]
    lagc1, lagc2 = prim["lagc1"], prim["lagc2"]
    x0, x1, x998, x999 = prim["x0"], prim["x1"], prim["x998"], prim["x999"]
    dom = prim["dom"]
    mean = S1 / L
    var = (S2 - S1 * S1 / L) / (L - 1)
    std = np.sqrt(np.maximum(var, 0))
    ptp = mx - mn
    zc = ((L - 1) - Sss) / 2 / (L - 1)
    energy = S2 / L
    rms = np.sqrt(np.maximum(S2 / L, 0))
    lag1 = lagc1 - x999 * x0
    lag2 = lagc2 - x998 * x0 - x999 * x1
    Sd = x999 - x0
    Sd2sum = 2 * S2 - x0 ** 2 - x999 ** 2 - 2 * lag1
    mean_slope = Sd / (L - 1)
    std_slope = np.sqrt(np.maximum(
        (Sd2sum - Sd ** 2 / (L - 1)) / (L - 2), 0))
    d0 = x1 - x0
    d998 = x999 - x998
    Sdd = 2 * lag1 - x0 * x1 - x998 * x999 - (S2 - x0 ** 2 - x999 ** 2) - lag2
    Sd2 = d998 - d0
    Sd2sq = 2 * Sd2sum - d0 ** 2 - d998 ** 2 - 2 * Sdd
    mean_curv = Sd2 / (L - 2)
    std_curv = np.sqrt(np.maximum(
        (Sd2sq - Sd2 ** 2 / (L - 2)) / (L - 3), 0))
    m3 = (S3 - 3 * mean * S2 + 2 * L * mean ** 3) / L
    m4 = (S4 - 4 * mean * S3 + 6 * mean ** 2 * S2 - 3 * L * mean ** 4) / L
    skew = m3 / (std ** 3 + EPS)
    kurt = m4 / (std ** 4 + EPS)
    itp = 1.0 / (tp + EPS)
    cent = Sf * itp
    bw = np.sqrt(np.maximum((Sf2 - 2 * cent * Sf + cent ** 2 * tp) * itp, 0))
    return np.stack([mean, std, var, ptp, zc, energy, rms,
                     mean_slope, std_slope, mean_curv, std_curv,
                     skew, kurt, dom, hr * itp, rr * itp, cent, bw],
                    axis=1).astype(np.float32)


def kernel(x):
    x = np.ascontiguousarray(np.asarray(x), dtype=np.float32)
    assert x.shape == (NCORE * RPC, L)
    (fn, consts) = _get_compiled()
    ps_g, st_g, acc_g = fn(x, consts)
    ps_g = np.asarray(ps_g).reshape(NCORE, 512, RPC)
    st_g = np.asarray(st_g).reshape(NCORE, 80, RPC)[:, :40]
    acc_g = np.asarray(acc_g).reshape(NCORE, 128, 128, 8)

    # per-bin index map: ps row (c//2)*128 + 64*(c%2) + i  -> bin chunk_bins(c)[i]
    rowmap = np.zeros(501, np.int32)
    for c in range(8):
        for i, k in enumerate(_chunk_bins(c)):
            rowmap[k] = (c // 2) * 128 + 64 * (c % 2) + i

    B = NCORE * RPC
    prim = {}
    # acc: [core, p, t, j]; row = core*RPC + t*128 + p
    accr = acc_g.transpose(0, 2, 1, 3).reshape(B, 8)
    prim["S3"] = accr[:, 0].astype(np.float64)
    prim["Sss"] = accr[:, 1].astype(np.float64)
    prim["min"] = accr[:, 2].astype(np.float64)
    prim["max"] = accr[:, 3].astype(np.float64)
    prim["S4"] = accr[:, 4].astype(np.float64)
    # st: [core, stat, row]
    st = st_g.transpose(0, 2, 1).reshape(B, 40).astype(np.float64)
    prim["tp"], prim["hr"], prim["rr"] = st[:, 0], st[:, 1], st[:, 2]
    prim["Sf"], prim["Sf2"] = st[:, 3], st[:, 4]
    prim["lagc1"], prim["lagc2"] = st[:, 5], st[:, 6]
    prim["x0"], prim["x1"] = st[:, 32], st[:, 33]
    prim["x998"], prim["x999"] = st[:, 34], st[:, 35]
    prim["S1"] = st[:, 36]
    # ps -> [B, 501]
    ps = ps_g.transpose(0, 2, 1).reshape(B, 512)[:, rowmap]
    prim["S2"] = (2 * prim["tp"] - ps[:, 0].astype(np.float64)
                  - ps[:, 500].astype(np.float64)) / L

    am = np.argmax(ps, axis=1)
    top2 = np.partition(ps, -2, axis=1)[:, -2:]
    gap = (top2[:, 1] - top2[:, 0]) / np.maximum(top2[:, 1], 1e-30)
    bad = np.nonzero(gap < 1e-2)[0]
    if len(bad):
        Xb = np.fft.rfft(x[bad].astype(np.float64), axis=1)
        am[bad] = np.argmax(np.abs(Xb) ** 2, axis=1)
    prim["dom"] = am * 0.1

    return _features_host(prim)

